# revision 96
# baseline (speedup 1.0000x reference)
"""Trainium2 Bass kernel for nn_Bottleneck_7911329759669 (topk_masking bottleneck).

Self-contained: builds the Bass module on first call, runs SPMD on 8 NeuronCores
(data-parallel over batch, 8 samples per core), returns the full output.

Per-sample pipeline (x: [256, 3136] fp32):
  - value path (conv1/conv2/conv3/identity/dilation) runs in bf16 on the PE
    (1 cycle/row); the spatial-saliency row runs in fp32 (4 cycles/row) because
    the exact top-k boundary cannot tolerate bf16/fp32r rounding.
  - x is loaded once in fp32 (saliency + pooling read it exactly); a bf16 copy
    xr is produced on the idle GpSimd engine, with the channel-pooling sum
    riding the same pass via accum_out.
  - channel top-32 mask: exact pairwise greater-counts on fp32 saliency.
  - spatial top-1568 mask: 32-step bitwise bisection on the sortable-u32
    transform; per-bit counts on DVE (fused or+cmp+accum STT), cross-partition
    reduction + threshold update on GpSimd (partition_all_reduce) so the PE
    queue never stalls on the bisection.
  - issue order is software-pipelined: A/bisect of quad q+1 issue before C of
    quad q so the bisection overlaps the next quad's conv1 work on the PE.
  - 3x3 mask dilation: K=9 ones-matmul over 9 shifted copies of the padded
    mask row (row 0 of the sh9 tile IS the mask row); applied as min(cnt,1)*r.
  - conv2 (3x3) as 6 accumulated matmuls on a row-padded layout (stride 58).
  - conv3 (1x1) as K=65 matmul: bn3 scale folded into weights, bn3 bias via
    the mask row (b3 (x) mask rank-1 term); identity added with a bf16
    eye-matmul; final ReLU on ScalarE during PSUM eviction into a per-sample
    staging buffer stored with one DMA.

mask_b is ignored: adding a constant to the saliency cannot change its top-k
mask, and the saliency itself is not part of the output.
"""
import sys

for _p in ("/opt/trn_rl_repo",):
    if _p not in sys.path:
        sys.path.insert(0, _p)

import numpy as np

import concourse.bass as bass
import concourse.bass_isa as bass_isa
import concourse.tile as tile
from concourse import bacc, mybir

F32 = mybir.dt.float32
BF16 = mybir.dt.bfloat16
U32 = mybir.dt.uint32
I32 = mybir.dt.int32
OP = mybir.AluOpType
AF = mybir.ActivationFunctionType
AX = mybir.AxisListType
RED = bass_isa.ReduceOp

B, CIN, H, W = 64, 256, 56, 56
WIDTH, COUT = 64, 256
N = H * W                      # 3136
K_SP, K_CH = 1568, 32
EPS = 1e-5
NCORES = 8
SPC = B // NCORES              # 8 samples per core

PW = W + 2                     # padded row stride
BASE = 64
NP = BASE + PW * H + BASE      # 3376
CH = 448                       # pixels per chunk (8 rows)
NCH = N // CH                  # 7
RPC = CH // W                  # 8 rows per chunk

UP, UF = 112, 28               # 112*28 == 3136
PAIR = 2

import os
DEBUG = bool(int(os.environ.get("KDEBUG", "0")))
KSKIP = set(os.environ.get("KSKIP", "").split(","))


def _padded(t, p0, p1, chunk, off):
    """[p1-p0, 8, 56] view of padded tile t at pixel chunk `chunk` shifted by off."""
    start = BASE + PW * RPC * chunk + off
    return t[p0:p1, start:start + PW * RPC].rearrange("p (h w) -> p h w", h=RPC)[:, :, 0:W]


def _build_nc():
    nc = bacc.Bacc("TRN2", target_bir_lowering=False, debug=False)

    x_d = nc.dram_tensor("x", [SPC, CIN, N], F32, kind="ExternalInput").ap()
    c1w_d = nc.dram_tensor("conv1_w", [WIDTH, CIN], F32, kind="ExternalInput").ap()
    bn1 = {k: nc.dram_tensor(f"bn1_{k}", [WIDTH], F32, kind="ExternalInput").ap() for k in "gbmv"}
    c2w_d = nc.dram_tensor("conv2_w", [WIDTH, WIDTH, 3, 3], F32, kind="ExternalInput").ap()
    bn2 = {k: nc.dram_tensor(f"bn2_{k}", [WIDTH], F32, kind="ExternalInput").ap() for k in "gbmv"}
    c3w_d = nc.dram_tensor("conv3_w", [COUT, WIDTH], F32, kind="ExternalInput").ap()
    bn3 = {k: nc.dram_tensor(f"bn3_{k}", [COUT], F32, kind="ExternalInput").ap() for k in "gbmv"}
    fcw_d = nc.dram_tensor("fc_w", [WIDTH, CIN], F32, kind="ExternalInput").ap()
    fcb_d = nc.dram_tensor("fc_b", [WIDTH], F32, kind="ExternalInput").ap()
    mw_d = nc.dram_tensor("mask_w", [CIN], F32, kind="ExternalInput").ap()
    nc.dram_tensor("mask_b", [1], F32, kind="ExternalInput")  # unused (constant shift)
    y_d = nc.dram_tensor("y", [SPC, COUT, N], F32, kind="ExternalOutput").ap()

    dbg = {}
    if DEBUG:
        dbg["sal"] = nc.dram_tensor("dbg_sal", [SPC, 64], F32, kind="ExternalOutput").ap()
        dbg["vec"] = nc.dram_tensor("dbg_vec", [SPC, 64], F32, kind="ExternalOutput").ap()
        dbg["sp"] = nc.dram_tensor("dbg_sp", [SPC, N], F32, kind="ExternalOutput").ap()
        dbg["u"] = nc.dram_tensor("dbg_u", [SPC, UP, UF], U32, kind="ExternalOutput").ap()
        dbg["lo"] = nc.dram_tensor("dbg_lo", [SPC, UP], U32, kind="ExternalOutput").ap()
        dbg["mask"] = nc.dram_tensor("dbg_mask", [SPC, N], F32, kind="ExternalOutput").ap()
        dbg["t12"] = nc.dram_tensor("dbg_t12", [SPC, 128, NP], F32, kind="ExternalOutput").ap()
        dbg["rhs65"] = nc.dram_tensor("dbg_rhs65", [SPC, 65, N], F32, kind="ExternalOutput").ap()

    np_bf16 = mybir.dt.np(BF16)
    eye128b_d = nc.inline_tensor(np.eye(128).astype(np_bf16), "eye128b").ap()
    eye64f_d = nc.inline_tensor(np.eye(64, dtype=np.float32), "eye64f").ap()
    ones1x64f_d = nc.inline_tensor(np.ones((1, 64), np.float32), "ones1x64f").ap()
    ones1x64b_d = nc.inline_tensor(np.ones((1, 64)).astype(np_bf16), "ones1x64b").ap()
    ones9b_d = nc.inline_tensor(np.ones((9, 64)).astype(np_bf16), "ones9b").ap()
    bits_np = np.zeros((UP, 65), np.uint32)
    bits_np[:, 0:32] = (np.uint32(1) << np.arange(32, dtype=np.uint32))[None, :]
    bits_np[:, 32] = 0x80000000
    bits_np[:, 33:65] = np.arange(32, dtype=np.uint32)[None, :]
    bits_d = nc.inline_tensor(bits_np, "bits_tbl").ap()

    consts_d = dict(eye128b=eye128b_d, eye64f=eye64f_d, ones1x64f=ones1x64f_d,
                    ones1x64b=ones1x64b_d, ones9b=ones9b_d, bits=bits_d)

    from contextlib import ExitStack
    with tile.TileContext(nc) as tc, ExitStack() as ctx:
        _body(ctx, tc, nc, x_d, y_d, c1w_d, bn1, c2w_d, bn2, c3w_d, bn3,
              fcw_d, fcb_d, mw_d, consts_d, dbg)
    nc.compile()
    return nc


def _body(ctx, tc, nc, x_d, y_d, c1w_d, bn1, c2w_d, bn2, c3w_d, bn3,
          fcw_d, fcb_d, mw_d, consts_d, dbg):
    consts = ctx.enter_context(tc.tile_pool(name="consts", bufs=1))
    xpool = ctx.enter_context(tc.tile_pool(name="xp", bufs=4))
    xrpool = ctx.enter_context(tc.tile_pool(name="xrp", bufs=6))
    statics = ctx.enter_context(tc.tile_pool(name="statics", bufs=4))
    rhs65p = ctx.enter_context(tc.tile_pool(name="rhs65", bufs=2))
    rowp = ctx.enter_context(tc.tile_pool(name="rows", bufs=1))
    smallp = ctx.enter_context(tc.tile_pool(name="smalls", bufs=4))
    stagep = ctx.enter_context(tc.tile_pool(name="stage", bufs=1))
    upool = ctx.enter_context(tc.tile_pool(name="utiles", bufs=5))
    outp = ctx.enter_context(tc.tile_pool(name="outs", bufs=2))
    ybufp = ctx.enter_context(tc.tile_pool(name="ybufp", bufs=1))
    ps_z1 = ctx.enter_context(tc.tile_pool(name="ps_z1", bufs=2, space="PSUM"))
    ps_z2 = ctx.enter_context(tc.tile_pool(name="ps_z2", bufs=2, space="PSUM"))
    ps_z3 = ctx.enter_context(tc.tile_pool(name="ps_z3", bufs=2, space="PSUM"))
    ps_cnt = ctx.enter_context(tc.tile_pool(name="ps_cnt", bufs=1, space="PSUM"))
    ps_sm = ctx.enter_context(tc.tile_pool(name="ps_sm", bufs=1, space="PSUM"))

    # ---------- first-quad x loads, ahead of the consts setup (SP/HWDGE are
    # in-order: issuing consts first would delay the loads that gate the
    # first-quad pipeline); sample 1 follows right after the consts
    preload = {}

    def _preload(s):
        for k in range(2):
            xt = xpool.tile([128, N], F32, name=f"x{k}_s{s}", tag="x")
            nc.sync.dma_start(xt, x_d[s, 128 * k:128 * (k + 1)])
            xr = xrpool.tile([128, N], BF16, name=f"xr{k}_s{s}", tag="xr")
            nc.scalar.copy(xr, xt)
            preload[(s, k)] = (xt, xr)

    _preload(0)
    _PRELOAD_ONLY_S0 = True

    # ---------- constants ----------
    ident = consts.tile([128, 128], BF16)
    nc.sync.dma_start(ident, consts_d["eye128b"])
    eye64f = consts.tile([64, 64], F32)
    nc.sync.dma_start(eye64f, consts_d["eye64f"])
    ones1x64f = consts.tile([1, 64], F32)
    nc.sync.dma_start(ones1x64f, consts_d["ones1x64f"])
    ones1x64 = consts.tile([1, 64], BF16)
    nc.sync.dma_start(ones1x64, consts_d["ones1x64b"])
    ones9 = consts.tile([9, 64], BF16)
    nc.sync.dma_start(ones9, consts_d["ones9b"])

    # u32 bit tables: cols 0..31 = 1<<k, col 32 = signbit, cols 33..64 = k
    bits = consts.tile([UP, 65], U32)
    nc.sync.dma_start(bits, consts_d["bits"])
    bitv = bits[:, 33:65]

    # conv1 lhsT: two [128, 64] K-tiles (bf16 via staged cast); saliency lhsT fp32
    w1, wsal = [], []
    for k in range(2):
        stg = stagep.tile([128, 64], F32, tag="wstg")
        nc.sync.dma_start(stg, c1w_d.transpose([1, 0])[128 * k:128 * (k + 1), :])
        t = consts.tile([128, 64], BF16, name=f"w1_{k}")
        nc.vector.tensor_copy(t, stg)
        w1.append(t)
        ws = consts.tile([128, 1], F32, name=f"wsal_{k}")
        nc.sync.dma_start(ws, mw_d[128 * k:128 * (k + 1)].unsqueeze(1))
        wsal.append(ws)

    # fc lhsT: two [128, 64] K-tiles; fc_b as [64,1]
    fcw = []
    for k in range(2):
        t = consts.tile([128, 64], F32, name=f"fcw_{k}")
        nc.sync.dma_start(t, fcw_d.transpose([1, 0])[128 * k:128 * (k + 1), :])
        fcw.append(t)
    fcb_col = consts.tile([64, 1], F32)
    nc.sync.dma_start(fcb_col, fcb_d.unsqueeze(1))

    # conv2 taps (bf16 via staged cast)
    def tap_ap(dy, dx):
        return c2w_d[:, :, dy + 1, dx + 1].transpose([1, 0])

    w2pair, w2sing = [], []
    for dy in (-1, 0, 1):
        stg = stagep.tile([128, 64], F32, tag="wstg")
        nc.sync.dma_start(stg[0:64], tap_ap(dy, -1))
        nc.sync.dma_start(stg[64:128], tap_ap(dy, 0))
        t = consts.tile([128, 64], BF16, name=f"w2p_{dy + 1}")
        nc.vector.tensor_copy(t, stg)
        w2pair.append(t)
        stg2 = stagep.tile([64, 64], F32, tag="wstg2")
        nc.sync.dma_start(stg2, tap_ap(dy, 1))
        s = consts.tile([64, 64], BF16, name=f"w2s_{dy + 1}")
        nc.vector.tensor_copy(s, stg2)
        w2sing.append(s)

    eps64 = consts.tile([64, 1], F32)
    nc.vector.memset(eps64, EPS)
    eps2 = consts.tile([2, 1], F32)
    nc.vector.memset(eps2, EPS)

    # bn1 / bn2 scale+bias columns [64,1]
    def bn_prep64(bnd, nm):
        cols = {}
        for k in "gbmv":
            c = smallp.tile([64, 1], F32, name=f"{nm}_{k}", tag=f"{nm}_{k}")
            nc.sync.dma_start(c, bnd[k].unsqueeze(1))
            cols[k] = c
        sd = smallp.tile([64, 1], F32, name=f"{nm}_sd", tag=f"{nm}_sd")
        nc.scalar.activation(sd, cols["v"], AF.Sqrt, bias=eps64, scale=1.0)
        rs = smallp.tile([64, 1], F32, name=f"{nm}_rs", tag=f"{nm}_rs")
        nc.vector.reciprocal(rs, sd)
        s = consts.tile([64, 1], F32, name=f"{nm}_s")
        nc.vector.tensor_mul(s, cols["g"], rs)
        bp = consts.tile([64, 1], F32, name=f"{nm}_bp")
        nc.vector.tensor_mul(bp, cols["m"], s)
        nc.vector.tensor_sub(bp, cols["b"], bp)
        return s, bp

    s1c, b1c = bn_prep64(bn1, "bn1")
    s2c, b2c = bn_prep64(bn2, "bn2")

    # bn3 in [2,128] layout (c = 128*p + f), then conv3 lhsT [65, 256] bf16
    def load_2x128(d, nm):
        t = smallp.tile([2, 128], F32, name=nm, tag=nm)
        nc.sync.dma_start(t, d.rearrange("(p f) -> p f", p=2))
        return t

    g3 = load_2x128(bn3["g"], "g3")
    b3 = load_2x128(bn3["b"], "b3")
    m3 = load_2x128(bn3["m"], "m3")
    v3 = load_2x128(bn3["v"], "v3")
    sd3 = smallp.tile([2, 128], F32, tag="sd3")
    nc.scalar.activation(sd3, v3, AF.Sqrt, bias=eps2, scale=1.0)
    rs3 = smallp.tile([2, 128], F32, tag="rs3")
    nc.vector.reciprocal(rs3, sd3)
    s3 = consts.tile([2, 128], F32)
    nc.vector.tensor_mul(s3, g3, rs3)
    b3p = consts.tile([2, 128], F32)
    nc.vector.tensor_mul(b3p, m3, s3)
    nc.vector.tensor_sub(b3p, b3, b3p)

    w3stg = consts.tile([64, 256], F32)
    nc.sync.dma_start(w3stg, c3w_d.transpose([1, 0]))
    s3row = consts.tile([1, 256], F32)
    nc.sync.dma_start(s3row, s3)          # [2,128] -> [1,256] partition-major
    b3row = consts.tile([1, 256], F32)
    nc.sync.dma_start(b3row, b3p)
    w3 = consts.tile([65, 256], BF16)
    nc.vector.tensor_copy(w3[64:65], b3row)
    s3b = ps_sm.tile([64, 256], F32, tag="sm")
    nc.tensor.matmul(s3b, ones1x64f, s3row, start=True, stop=True)
    nc.vector.tensor_mul(w3[0:64], w3stg, s3b)   # bf16 out: w3 * s3 fold

    # padded t12 statics, 4-deep ring for the software pipeline (pads zeroed once)
    t12s = []
    for i in range(4):
        t = statics.tile([128, NP], BF16, name=f"t12_{i}", tag="t12")
        nc.vector.memset(t, 0.0)
        t12s.append(t)
    sh9s = []
    for i in range(2):
        t = rowp.tile([9, NP], BF16, name=f"sh9_{i}", tag=f"sh9_{i}")
        nc.vector.memset(t.bitcast(F32), 0.0)   # pads + edge strips must read 0
        sh9s.append(t)
    sprow0 = rowp.tile([1, N], F32, name="sprow0", tag="sprow")
    DELTAS = [dy * PW + dx for dy in (-1, 0, 1) for dx in (-1, 0, 1)]

    class S:
        pass

    # ---------------- stage A ----------------
    def stage_a(s):
        st = S()
        st.x, st.xr, st.pool = [], [], []
        for k in range(2):
            if (s, k) in preload:
                xt, xr = preload[(s, k)]
            else:
                xt = xpool.tile([128, N], F32, name=f"x{k}_s{s}", tag="x")
                nc.sync.dma_start(xt, x_d[s, 128 * k:128 * (k + 1)])
                # bf16 copy for the PE value path (Act queue: never blocked
                # by the bisection, unlike Pool/DVE)
                xr = xrpool.tile([128, N], BF16, name=f"xr{k}_s{s}", tag="xr")
                nc.scalar.copy(xr, xt)
            st.x.append(xt)
            st.xr.append(xr)
            pool = smallp.tile([128, 1], F32, tag=f"pool{k}")
            nc.vector.reduce_sum(pool, xt, axis=AX.X)
            st.pool.append(pool)
        if "a1" in KSKIP:
            st.s2v, st.b2v, st.w2p, st.w2s = s2c, b2c, w2pair, w2sing
        st.sprow = None

        if "a2" not in KSKIP:
            _stage_a2(st, s)
        if "a1" not in KSKIP:
            _stage_a1(st, s)
        if "a3" not in KSKIP:
            _stage_a3(st, s)
        return st

    def _stage_a1(st, s):
        if "a1x" in KSKIP:
            st.s2v, st.b2v, st.w2p, st.w2s = s2c, b2c, w2pair, w2sing
        fcps = ps_sm.tile([64, 1], F32, tag="sm")
        nc.tensor.matmul(fcps, fcw[0], st.pool[0], start=True, stop=False)
        nc.tensor.matmul(fcps, fcw[1], st.pool[1], start=False, stop=True)
        sal = smallp.tile([64, 1], F32, tag="sal")
        nc.scalar.activation(sal, fcps, AF.Sigmoid, bias=fcb_col, scale=1.0 / N)
        if "a1x" in KSKIP:
            return
        salT = ps_sm.tile([1, 64], F32, tag="sm")
        nc.tensor.transpose(salT, sal, eye64f)
        salrow = smallp.tile([1, 64], F32, tag="salrow")
        nc.scalar.copy(salrow, salT)
        if "a1y" in KSKIP:
            st.s2v, st.b2v, st.w2p, st.w2s = s2c, b2c, w2pair, w2sing
            return
        aps = ps_sm.tile([64, 64], F32, tag="sm")
        nc.tensor.matmul(aps, ones1x64f, salrow, start=True, stop=True)
        scr = smallp.tile([64, 64], F32, tag="scr")
        cnt = smallp.tile([64, 1], F32, tag="cnt")
        # in1 must be SBUF: DVE has a single PSUM read port (in0=aps is PSUM)
        nc.vector.scalar_tensor_tensor(scr, aps, sal, sal.broadcast_to([64, 64]),
                                       op0=OP.is_gt, op1=OP.bypass, accum_out=cnt)
        if "a1z" in KSKIP:
            st.s2v, st.b2v, st.w2p, st.w2s = s2c, b2c, w2pair, w2sing
            return
        vec = smallp.tile([64, 1], F32, tag="vec")
        nc.vector.tensor_scalar(vec, cnt, float(K_CH), None, op0=OP.is_lt)
        if DEBUG:
            nc.sync.dma_start(dbg["sal"][s], sal)
            nc.sync.dma_start(dbg["vec"][s], vec)
        st.s2v = smallp.tile([64, 1], F32, tag="s2v")
        nc.vector.tensor_mul(st.s2v, s2c, vec)
        st.b2v = smallp.tile([64, 1], F32, tag="b2v")
        nc.vector.tensor_mul(st.b2v, b2c, vec)
        st.w2p, st.w2s = [], []
        for i in range(3):
            wp = smallp.tile([128, 64], BF16, tag=f"w2vp{i}")
            nc.vector.tensor_scalar(wp[0:64], w2pair[i][0:64], vec, None, op0=OP.mult)
            nc.vector.tensor_scalar(wp[64:128], w2pair[i][64:128], vec, None, op0=OP.mult)
            st.w2p.append(wp)
            ws = smallp.tile([64, 64], BF16, tag=f"w2vs{i}")
            nc.vector.tensor_scalar(ws, w2sing[i], vec, None, op0=OP.mult)
            st.w2s.append(ws)

    def _stage_a2(st, s):
        st.t12 = t12s[s % 4]
        sprow = sprow0
        for c in range(NCH):
            z1 = ps_z1.tile([65, CH], F32, tag="z1")
            nc.tensor.matmul(z1[0:64], w1[0], st.xr[0][:, c * CH:(c + 1) * CH], start=True, stop=False)
            nc.tensor.matmul(z1[0:64], w1[1], st.xr[1][:, c * CH:(c + 1) * CH], start=False, stop=True)
            nc.tensor.matmul(z1[64:65], wsal[0], st.x[0][:, c * CH:(c + 1) * CH],
                             start=True, stop=False, skip_group_check=True)
            nc.tensor.matmul(z1[64:65], wsal[1], st.x[1][:, c * CH:(c + 1) * CH],
                             start=False, stop=True, skip_group_check=True)
            tv = _padded(st.t12, 0, 64, c, 0)
            zv = z1[0:64].rearrange("p (h w) -> p h w", h=RPC)
            nc.scalar.activation(tv, zv, AF.Relu, bias=b1c, scale=s1c)
            nc.scalar.copy(sprow[:, c * CH:(c + 1) * CH], z1[64:65])

        st.sprow = sprow

    def _stage_a3(st, s):
        sprow = st.sprow
        # sortable-u32 transform: u = bits ^ ((bits >> 31) | 0x80000000)
        st.u = upool.tile([UP, UF], U32, name=f"u_s{s}", tag="u")
        nc.sync.dma_start(st.u.bitcast(F32), sprow)
        if DEBUG:
            nc.sync.dma_start(dbg["sp"][s], sprow)
        bb = upool.tile([UP, UF], U32, tag="bb")
        nc.vector.tensor_scalar(bb.bitcast(I32), st.u.bitcast(I32),
                                31, None, op0=OP.arith_shift_right)
        nc.vector.tensor_tensor(bb, bb, bits[:, 32:33].broadcast_to([UP, UF]),
                                op=OP.bitwise_or)
        nc.vector.tensor_tensor(st.u, st.u, bb, op=OP.bitwise_xor)
        if DEBUG:
            nc.sync.dma_start(dbg["u"][s], st.u)

    # ---------------- bisection (PAIR samples) ----------------
    def bisect(quad, q):
        lo = upool.tile([UP, PAIR], U32, name=f"lo_q{q}", tag="lo")
        nc.vector.memset(lo, 0)
        csum = upool.tile([UP, PAIR], F32, tag="csum")
        cntq = upool.tile([UP, PAIR], F32, tag="cntq")
        flag = upool.tile([UP, PAIR], U32, tag="flag")
        scr = upool.tile([UP, UF], F32, tag="uscr")
        mt = upool.tile([UP, PAIR], U32, tag="mt")
        for bit in range(31, -1, -1):
            if bit == 31:
                mtv = bits[:, 32:33].broadcast_to([UP, PAIR])
            else:
                nc.vector.tensor_tensor(mt, lo, bits[:, bit:bit + 1].broadcast_to([UP, PAIR]),
                                        op=OP.bitwise_or)
                mtv = mt
            for i, st in enumerate(quad):
                nc.vector.scalar_tensor_tensor(
                    scr, st.u, 0, mtv[:, i:i + 1].broadcast_to([UP, UF]),
                    op0=OP.bypass, op1=OP.is_gt, accum_out=csum[:, i:i + 1])
            # cross-partition total on GpSimd: PE untouched
            nc.gpsimd.partition_all_reduce(cntq, csum, UP, RED.add)
            # flag = (cnt >= K) * 2^bit (arith+arith fusion), lo |= flag
            nc.vector.scalar_tensor_tensor(flag, cntq, float(K_SP),
                                           bits[:, bit:bit + 1].broadcast_to([UP, PAIR]),
                                           op0=OP.is_ge, op1=OP.mult)
            nc.vector.tensor_tensor(lo, lo, flag, op=OP.bitwise_or)
        for i, st in enumerate(quad):
            st.lo, st.lo_i = lo, i
            if DEBUG:
                nc.sync.dma_start(dbg["lo"][q * PAIR + i], lo[:, i:i + 1])

    # ---------------- stage C ----------------
    def stage_c_mask(s, st):
        mtile = upool.tile([UP, UF], BF16, tag="mask")
        nc.vector.tensor_tensor(mtile, st.u,
                                st.lo[:, st.lo_i:st.lo_i + 1].broadcast_to([UP, UF]),
                                op=OP.is_gt)
        rhs65 = rhs65p.tile([65, N], BF16, tag="rhs65")
        nc.sync.dma_start(rhs65[64:65], mtile)
        sh9 = sh9s[s % 2]
        mrow = sh9[0:1]          # row 0 = unshifted mask (mbc needs base partition 0)
        mpad = mrow[:, BASE:BASE + PW * H].rearrange("p (h w) -> p h w", h=H)[:, :, 0:W]
        nc.scalar.dma_start(mpad, rhs65[64:65])
        th = sh9[0:1].tensor
        # grouped shift DMAs from row 0: rows 1-3 = dy=-1 (dx -1,0,1), rows 4,5 =
        # (0,-1),(0,+1), rows 6-8 = dy=+1
        WIN = PW * H
        g_m1_src = bass.AP(tensor=th, offset=BASE - PW - 1, ap=[[NP, 1], [1, 3], [1, WIN]])
        g_m1_dst = bass.AP(tensor=th, offset=1 * NP + BASE, ap=[[NP, 3], [1, WIN]])
        nc.sync.dma_start(g_m1_dst, g_m1_src)
        g_0_src = bass.AP(tensor=th, offset=BASE - 1, ap=[[NP, 1], [2, 2], [1, WIN]])
        g_0_dst = bass.AP(tensor=th, offset=4 * NP + BASE, ap=[[NP, 2], [1, WIN]])
        nc.scalar.dma_start(g_0_dst, g_0_src)
        g_p1_src = bass.AP(tensor=th, offset=BASE + PW - 1, ap=[[NP, 1], [1, 3], [1, WIN]])
        g_p1_dst = bass.AP(tensor=th, offset=6 * NP + BASE, ap=[[NP, 3], [1, WIN]])
        nc.sync.dma_start(g_p1_dst, g_p1_src)
        if DEBUG:
            nc.gpsimd.dma_start(dbg["mask"][s], rhs65[64:65])
        st.rhs65, st.sh9 = rhs65, sh9

    def stage_c_rest(s, st):
        rhs65, sh9 = st.rhs65, st.sh9
        mrow = sh9[0:1]
        t12 = st.t12
        for c in range(NCH):
            cnt9 = ps_cnt.tile([64, CH], F32, tag="cnt")
            nc.tensor.matmul(cnt9, ones9,
                             _padded(sh9, 0, 9, c, 0), start=True, stop=True)
            tv = _padded(t12, 0, 64, c, 0)
            cv = cnt9.rearrange("p (h w) -> p h w", h=RPC)
            nc.vector.scalar_tensor_tensor(tv, cv, 1.0, tv, op0=OP.min, op1=OP.mult)
        # pre-shifted copy for the dx=(-1,0) tap pairs (cross-partition => DMA)
        nc.scalar.dma_start(t12[64:128, 0:NP - 1], t12[0:64, 1:NP])
        if DEBUG:
            nc.gpsimd.dma_start(dbg["t12"][s], t12)

        for c in range(NCH):
            z2 = ps_z2.tile([64, CH], F32, tag="z2")
            for i, dy in enumerate((-1, 0, 1)):
                nc.tensor.matmul(z2, st.w2p[i],
                                 _padded(t12, 0, 128, c, PW * dy - 1),
                                 start=(i == 0), stop=False)
                nc.tensor.matmul(z2, st.w2s[i],
                                 _padded(t12, 0, 64, c, PW * dy + 1),
                                 start=False, stop=(i == 2))
            r2 = outp.tile([64, CH], BF16, tag="r2")
            nc.scalar.activation(r2, z2, AF.Relu, bias=st.b2v, scale=st.s2v)
            mbc = ps_cnt.tile([64, CH], F32, tag="cnt")
            nc.tensor.matmul(mbc, ones1x64,
                             _padded(sh9, 0, 1, c, 0),
                             start=True, stop=True)
            nc.vector.scalar_tensor_tensor(rhs65[0:64, c * CH:(c + 1) * CH],
                                           mbc, 1.0, r2, op0=OP.bypass, op1=OP.mult)

        if DEBUG:
            nc.gpsimd.dma_start(dbg["rhs65"][s], rhs65)
        ybuf = ybufp.tile([128, 2, N], F32, tag="ybuf")
        for c in range(NCH):
            for m in range(2):
                z3 = ps_z3.tile([128, CH], F32, tag="z3")
                nc.tensor.matmul(z3, w3[:, 128 * m:128 * (m + 1)],
                                 rhs65[:, c * CH:(c + 1) * CH], start=True, stop=False)
                nc.tensor.matmul(z3, ident, st.xr[m][:, c * CH:(c + 1) * CH],
                                 start=False, stop=True)
                nc.scalar.activation(ybuf[:, m, c * CH:(c + 1) * CH], z3, AF.Relu)
        # one batched store: [128, m, n] -> y[s, m*128 + p, n]
        nc.sync.dma_start(y_d[s].rearrange("(m p) n -> p m n", p=128), ybuf)

    # software pipeline: A/bisect of quad q issue before C of quad q-1, so the
    # bisection overlaps the next quad's conv1/saliency PE work.
    prev = []
    for q in range(SPC // PAIR):
        if "c" not in KSKIP:
            for ss, st in prev:
                stage_c_mask(ss, st)
        sts = [stage_a(q * PAIR + i) for i in range(PAIR)]
        if "c" not in KSKIP:
            for ss, st in prev:
                stage_c_rest(ss, st)
        if "b" not in KSKIP:
            bisect(sts, q)
        else:
            lo = upool.tile([UP, PAIR], U32, tag="lo")
            nc.vector.memset(lo, 0)
            for i, st in enumerate(sts):
                st.lo, st.lo_i = lo, i
        prev = [(q * PAIR + i, sts[i]) for i in range(PAIR)]
    if "c" not in KSKIP:
        for ss, st in prev:
            stage_c_mask(ss, st)
            stage_c_rest(ss, st)


_CACHED = {}
LAST_RESULTS = None


def _get_nc():
    if "nc" not in _CACHED:
        _CACHED["nc"] = _build_nc()
    return _CACHED["nc"]


def kernel(**inputs):
    from concourse.bass_utils import run_bass_kernel_spmd
    nc = _get_nc()
    x = np.ascontiguousarray(np.asarray(inputs["x"], np.float32).reshape(B, CIN, N))
    base = {
        "conv1_w": np.ascontiguousarray(np.asarray(inputs["conv1_w"], np.float32).reshape(WIDTH, CIN)),
        "conv2_w": np.ascontiguousarray(np.asarray(inputs["conv2_w"], np.float32)),
        "conv3_w": np.ascontiguousarray(np.asarray(inputs["conv3_w"], np.float32).reshape(COUT, WIDTH)),
        "fc_w": np.ascontiguousarray(np.asarray(inputs["fc_w"], np.float32)),
        "fc_b": np.ascontiguousarray(np.asarray(inputs["fc_b"], np.float32)),
        "mask_w": np.ascontiguousarray(np.asarray(inputs["mask_w"], np.float32).reshape(CIN)),
        "mask_b": np.ascontiguousarray(np.asarray(inputs["mask_b"], np.float32)),
    }
    for pre in ("bn1", "bn2", "bn3"):
        for k in "gbmv":
            base[f"{pre}_{k}"] = np.ascontiguousarray(np.asarray(inputs[f"{pre}_{k}"], np.float32))
    in_maps = []
    for c in range(NCORES):
        m = dict(base)
        m["x"] = np.ascontiguousarray(x[c * SPC:(c + 1) * SPC])
        in_maps.append(m)
    res = run_bass_kernel_spmd(nc, in_maps, core_ids=list(range(NCORES)))
    global LAST_RESULTS
    LAST_RESULTS = res
    y = np.concatenate([r["y"] for r in res.results], axis=0)
    return y.reshape(B, COUT, H, W)
